# revision 28
# baseline (speedup 1.0000x reference)
"""Trainium2 Bass kernel for ContextQueryAttention (BiDAF-style trilinear attention).

Math (per batch b):
  S[n,m] = ctx[n]·w_c + q[m]·w_q + (ctx[n]*w_m)·q[m]
  A  = softmax_m(S + qmask_bias)      (bias -inf on masked m)
  Bm = softmax_n(S + cmask_bias)
  c2q = A @ q ;  q2c = A @ Bm^T @ ctx
  out = concat([ctx, c2q, ctx*c2q, ctx*q2c], -1)

On-chip decomposition (per core, 4 batches), with T = trilinear part only:
  ET[m,n]  = exp(T^T)                  computed directly in m-major via
                                       S^T = (q*w_m)^T-stationary @ ctx^T
  qs[m,:]  = expqb[m] * [q | 1]        expqb = exp(q@w_q + qmask_add), host-folded
  czc[n]   = c_mask[n] * exp(ctx@w_c)  host-folded (w_c term cancels in A-path,
                                       enters B-path only through this factor)
  A-path:  c2q_raw[n,:] = ET^T @ qs    -> rowsum' in last col
  B-path:  Em = czc * E (fused into the ET->E transpose copy)
           C1raw[m,:] = Em^T @ [ctx | 1] -> colsum in last col
           C1s = (expqb/colsum) * C1raw
           q2c_raw = ET^T @ C1s
  c2q = c2q_raw / rowsum' ; q2c = q2c_raw / rowsum'   (softmax shifts cancel)

All matmuls run in bf16 (full PE rate, f32 PSUM).  Host pre-transposes
(q*w_m)^T and ctx^T, so the only on-chip transposes are ET->Em (16/batch).
All four input tensors are packed into ONE per-batch DMA (batch 0 splits it
into four so compute can start early).  Device ships normalized c2q|q2c in
bf16; the host assembles [ctx, c2q, ctx*c2q, ctx*q2c] in f32 during the
unshard (ctx block is exact).
Sharding: batch data-parallel, 4 of 32 batches per NeuronCore, 8 cores.
"""

import numpy as np
import ml_dtypes

BF16 = ml_dtypes.bfloat16

B, N, M, D = 32, 1024, 256, 512
NCORES = 8
BL = B // NCORES          # batches per core
NT = N // 128             # 8 context row tiles
MT = M // 128             # 2 query row tiles
DC = D // 128             # 4 feature chunks
NEG = -30000.0            # additive mask; exp(x + NEG) underflows to exactly 0.0

# column offsets inside the packed per-batch input block [128, MEGA]
OT = 0                    # ctxT   [p, dc*1024 + n]          (4096)
OW = OT + DC * N          # qTw    [p, dc*256 + m]           (1024)
OS = OW + DC * M          # qs     [p, mt*514 + d]           (1028)
OC = OS + MT * 514        # ctx    [p, nt*514 + d]           (4112)
MEGA = OC + NT * 514      # 10260

_built = {}


def _build_nc(repeat=1):
    import concourse.bass as bass  # noqa: F401
    import concourse.mybir as mybir
    import concourse.tile as tile
    from concourse import bacc
    from concourse.masks import make_identity

    f32 = mybir.dt.float32
    bf16 = mybir.dt.bfloat16
    EXP = mybir.ActivationFunctionType.Exp
    RECIP = mybir.ActivationFunctionType.Reciprocal
    MUL = mybir.AluOpType.mult

    nc = bacc.Bacc("TRN2", target_bir_lowering=False, debug=False)
    ctxT_d = nc.dram_tensor("ctxT", (BL, 128, DC * N), bf16, kind="ExternalInput")
    qTw_d = nc.dram_tensor("qTw", (BL, 128, DC * M), bf16, kind="ExternalInput")
    qs_d = nc.dram_tensor("qs", (BL, 128, MT * 514), bf16, kind="ExternalInput")
    ctx_d = nc.dram_tensor("ctx", (BL, 128, NT * 514), bf16, kind="ExternalInput")
    aux_d = nc.dram_tensor("aux", (128, 40), f32, kind="ExternalInput")
    out_d = nc.dram_tensor("out", (BL, NT, 128, 1024), bf16, kind="ExternalOutput")

    ctxT_ap = ctxT_d.ap()
    qTw_ap = qTw_d.ap()
    qs_ap = qs_d.ap()
    ctx_ap = ctx_d.ap()
    aux_ap = aux_d.ap()
    outv = out_d.ap()

    with tile.TileContext(nc) as tc:
        with (
            tc.tile_pool(name="singles", bufs=1) as singles,
            tc.tile_pool(name="p_ctxT", bufs=2) as p_ctxT,
            tc.tile_pool(name="p_qTw", bufs=2) as p_qTw,
            tc.tile_pool(name="p_qs", bufs=2) as p_qs,
            tc.tile_pool(name="p_ctx", bufs=2) as p_ctx,
            tc.tile_pool(name="p_et", bufs=2) as p_et,
            tc.tile_pool(name="p_em", bufs=2) as p_em,
            tc.tile_pool(name="p_c1", bufs=2) as p_c1,
            tc.tile_pool(name="p_small", bufs=2) as p_small,
            tc.tile_pool(name="p_out", bufs=2) as p_out,
            tc.tile_pool(name="ps2", bufs=2, space="PSUM") as ps2,
            tc.tile_pool(name="ps1", bufs=5, space="PSUM") as ps1,
            tc.tile_pool(name="psr", bufs=1, space="PSUM") as psr,
        ):
            aux_sb = singles.tile([128, 40], f32)
            id32 = singles.tile([128, 128], f32)
            make_identity(nc, id32)
            idb = singles.tile([128, 128], bf16)
            nc.vector.tensor_copy(idb, id32)

            n_iters = repeat * BL
            for it in range(n_iters):
                b = it % BL
                czc = aux_sb[:, b * 8:(b + 1) * 8]           # czc [128, NT]
                eqb = aux_sb[:, 32 + b * 2:32 + b * 2 + 2]   # expqb [128, MT]

                # ---- per-tensor input DMAs on the sync queue (prefetch a
                # full batch ahead).  Batch 0 splits ctxT into nh halves so
                # the first S^T group starts as early as possible.
                qTw_sb = p_qTw.tile([128, DC * M], bf16, tag="qTw")
                ctxT_sb = p_ctxT.tile([128, DC * N], bf16, tag="ctxT")
                if it == 0:
                    # fine-grained chunks so the first S^T group starts ~2.5us
                    # in, chasing the DMA instead of waiting for full tensors
                    qTw_src = qTw_ap[b].rearrange("p (dc m) -> p dc m", dc=DC)
                    qTw_dst = qTw_sb.rearrange("p (dc m) -> p dc m", dc=DC)
                    ctxT_src = ctxT_ap[b].rearrange("p (dc n) -> p dc n", dc=DC)
                    ctxT_dst = ctxT_sb.rearrange("p (dc n) -> p dc n", dc=DC)
                    nc.sync.dma_start(qTw_dst[:, :, 0:128], qTw_src[:, :, 0:128])
                    nc.sync.dma_start(ctxT_dst[:, 0:2, 0:512], ctxT_src[:, 0:2, 0:512])
                    nc.sync.dma_start(ctxT_dst[:, 2:4, 0:512], ctxT_src[:, 2:4, 0:512])
                    nc.sync.dma_start(qTw_dst[:, :, 128:256], qTw_src[:, :, 128:256])
                    nc.sync.dma_start(ctxT_dst[:, :, 512:1024], ctxT_src[:, :, 512:1024])
                    # aux (czc/expqb) is not needed until the Em phase; keep
                    # it off the critical batch-0 input chain
                    nc.sync.dma_start(aux_sb, aux_ap)
                else:
                    nc.sync.dma_start(qTw_sb, qTw_ap[b])
                    nc.sync.dma_start(ctxT_sb, ctxT_ap[b])
                qs_sb = p_qs.tile([128, MT * 514], bf16, tag="qs")
                nc.sync.dma_start(qs_sb, qs_ap[b])
                ctx_sb = p_ctx.tile([128, NT * 514], bf16, tag="ctx")
                nc.sync.dma_start(ctx_sb, ctx_ap[b])

                def qtw(dc, mt):
                    return qTw_sb[:, dc * 256 + mt * 128:dc * 256 + (mt + 1) * 128]

                def ctxT(dc, nh):
                    return ctxT_sb[:, dc * 1024 + nh * 512:dc * 1024 + (nh + 1) * 512]

                def qsv(mt, d0, d1):
                    return qs_sb[:, mt * 514 + d0:mt * 514 + d1]

                def ctxv(nt, d0, d1):
                    return ctx_sb[:, nt * 514 + d0:nt * 514 + d1]

                # ---- S^T matmuls + ET = exp(S^T), m-major (native A-path).
                # nh0 groups first so the ET(half0) transposes can start
                # right after the S^T phase.
                ET = p_et.tile([128, MT, 1024], bf16, tag="ET")
                for nh in range(2):
                    for mt in range(MT):
                        st_ps = ps1.tile([128, 512], f32, tag="ps1")
                        for dc in range(DC):
                            nc.tensor.matmul(
                                st_ps,
                                qtw(dc, mt),
                                ctxT(dc, nh),
                                start=(dc == 0), stop=(dc == DC - 1),
                            )
                        nc.scalar.activation(
                            ET[:, mt, nh * 512:(nh + 1) * 512], st_ps, EXP,
                        )

                # ---- ET -> Em transposes (PE) with czc scale fused in the
                # PSUM->SBUF copies (DVE), interleaved with the c2q subphase
                # so the PE never waits on the trailing exp or the copies.
                Em = p_em.tile([128, NT, 256], bf16, tag="Em")
                out_sb = p_out.tile([128, NT, 1024], bf16, tag="out_sb")
                rA = p_small.tile([128, NT], f32, tag="rA")
                sums_ps = psr.tile([128, 2 * NT + 2 * MT], f32, tag="psr")

                def et_transposes(half):
                    etp = ps2.tile([128, 1024], bf16, tag="ps2")
                    for k in range(4):
                        nt = half * 4 + k
                        for mt in range(MT):
                            nc.tensor.transpose(
                                etp[:, k * 256 + mt * 128:k * 256 + (mt + 1) * 128],
                                ET[:, mt, nt * 128:(nt + 1) * 128],
                                idb,
                            )
                    for k in range(4):
                        nt = half * 4 + k
                        nc.vector.tensor_scalar(
                            Em[:, nt, :], etp[:, k * 256:(k + 1) * 256],
                            czc[:, nt:nt + 1], None, MUL,
                        )

                def c2q_nt(nt):
                    c2q_ps = ps1.tile([128, 512], f32, tag="ps1")
                    rows = sums_ps[:, 2 * nt:2 * nt + 2]
                    for mt in range(MT):
                        nc.tensor.matmul(
                            c2q_ps,
                            ET[:, mt, nt * 128:(nt + 1) * 128],
                            qsv(mt, 0, 512),
                            start=(mt == 0), stop=(mt == MT - 1),
                        )
                        nc.tensor.matmul(
                            rows,
                            ET[:, mt, nt * 128:(nt + 1) * 128],
                            qsv(mt, 512, 514),
                            start=(mt == 0), stop=(mt == MT - 1),
                        )
                    nc.vector.reciprocal(rA[:, nt:nt + 1], rows[:, 0:1])
                    if nt % 2 == 0:
                        nc.scalar.mul(out_sb[:, nt, 0:512], c2q_ps, rA[:, nt:nt + 1])
                    else:
                        nc.vector.tensor_scalar(
                            out_sb[:, nt, 0:512], c2q_ps, rA[:, nt:nt + 1], None, MUL,
                        )

                et_transposes(0)
                et_transposes(1)
                for nt in range(NT):
                    c2q_nt(nt)
                # c2q output halves: two 4-nt DMAs, transfer during C1/q2c
                for g in range(2):
                    nc.gpsimd.dma_start(
                        outv[b, g * 4:(g + 1) * 4, :, 0:512].rearrange(
                            "nt p f -> p nt f"),
                        out_sb[:, g * 4:(g + 1) * 4, 0:512],
                    )

                # ---- C1 = Em^T @ [ctx | 1] (+colsum), scaled -> C1s (DVE)
                C1s = p_c1.tile([128, MT, 512], bf16, tag="C1s")
                rc = p_small.tile([128, MT], f32, tag="rc")
                rr = p_small.tile([128, MT], f32, tag="rr")
                for mt in range(MT):
                    c1_ps = ps2.tile([128, 512], f32, tag="ps2")
                    cols = sums_ps[:, 2 * NT + 2 * mt:2 * NT + 2 * mt + 2]
                    for nt in range(NT):
                        nc.tensor.matmul(
                            c1_ps,
                            Em[:, nt, mt * 128:(mt + 1) * 128],
                            ctxv(nt, 0, 512),
                            start=(nt == 0), stop=(nt == NT - 1),
                        )
                        nc.tensor.matmul(
                            cols,
                            Em[:, nt, mt * 128:(mt + 1) * 128],
                            ctxv(nt, 512, 514),
                            start=(nt == 0), stop=(nt == NT - 1),
                        )
                    nc.vector.reciprocal(rc[:, mt:mt + 1], cols[:, 0:1])
                    nc.vector.tensor_tensor(
                        rr[:, mt:mt + 1], rc[:, mt:mt + 1],
                        eqb[:, mt:mt + 1], MUL,
                    )
                    nc.vector.tensor_scalar(
                        C1s[:, mt, :], c1_ps,
                        rr[:, mt:mt + 1], None, MUL,
                    )

                # ---- q2c subphase in mt-passes over 2-nt groups: the mt0
                # passes need only C1s(mt0), covering the C1s(mt1) scale
                # latency on DVE.  2-nt output DMAs, trailing pair on the
                # Activation HWDGE queue for a fast drain.
                def q2c_pass(g, mt, tiles):
                    for i in range(2):
                        nt = g * 2 + i
                        nc.tensor.matmul(
                            tiles[i],
                            ET[:, mt, nt * 128:(nt + 1) * 128],
                            C1s[:, mt, :],
                            start=(mt == 0), stop=(mt == MT - 1),
                        )

                def q2c_finish(g, tiles):
                    for i in range(2):
                        nt = g * 2 + i
                        if nt % 2 == 0:
                            nc.scalar.mul(
                                out_sb[:, nt, 512:1024], tiles[i], rA[:, nt:nt + 1])
                        else:
                            nc.vector.tensor_scalar(
                                out_sb[:, nt, 512:1024], tiles[i],
                                rA[:, nt:nt + 1], None, MUL,
                            )
                    eng = nc.gpsimd if g < 2 else nc.scalar
                    eng.dma_start(
                        outv[b, g * 2:g * 2 + 2, :, 512:1024].rearrange(
                            "nt p f -> p nt f"),
                        out_sb[:, g * 2:g * 2 + 2, 512:1024],
                    )

                for gp in range(2):
                    ta0 = ps1.tile([128, 512], f32, tag="ps1")
                    ta1 = ps1.tile([128, 512], f32, tag="ps1")
                    tb0 = ps1.tile([128, 512], f32, tag="ps1")
                    tb1 = ps1.tile([128, 512], f32, tag="ps1")
                    ta = [ta0, ta1]
                    tb = [tb0, tb1]
                    ga, gb = 2 * gp, 2 * gp + 1
                    q2c_pass(ga, 0, ta)
                    q2c_pass(gb, 0, tb)
                    q2c_pass(ga, 1, ta)
                    q2c_finish(ga, ta)
                    q2c_pass(gb, 1, tb)
                    q2c_finish(gb, tb)

    nc.compile()
    return nc


def get_nc(repeat=1):
    key = ("nc", repeat)
    if key not in _built:
        _built[key] = _build_nc(repeat)
    return _built[key]


def _host_prep(context, query, c_mask, q_mask, w):
    context = np.ascontiguousarray(np.asarray(context, dtype=np.float32))
    query = np.ascontiguousarray(np.asarray(query, dtype=np.float32))
    c_mask = np.asarray(c_mask, dtype=bool)
    q_mask = np.asarray(q_mask, dtype=bool)
    w = np.asarray(w, dtype=np.float32).reshape(3 * D)
    w_q, w_c, w_m = w[0:D], w[D:2 * D], w[2 * D:]

    # host-folded softmax pieces (tiny matvecs)
    czc = (c_mask * np.exp(context @ w_c)).astype(np.float32)          # [B, N]
    expqb = np.exp(query @ w_q + np.where(q_mask, 0.0, NEG)).astype(np.float32)  # [B, M]

    mega = np.empty((B, 128, MEGA), dtype=BF16)
    ctx_bf = context.astype(BF16)
    # ctxT [p, dc*1024 + n] = ctx[n, dc*128 + p]
    mega[:, :, OT:OT + DC * N] = (
        ctx_bf.reshape(B, N, DC, 128).transpose(0, 3, 2, 1).reshape(B, 128, DC * N)
    )
    # qTw [p, dc*256 + m] = (q*w_m)[m, dc*128 + p]
    mega[:, :, OW:OW + DC * M] = (
        (query * w_m).astype(BF16).reshape(B, M, DC, 128).transpose(0, 3, 2, 1)
        .reshape(B, 128, DC * M)
    )
    # qs [p, mt*514 + d] = (expqb*[q | 1])[mt*128 + p, d]
    qs_p = np.empty((B, M, 514), dtype=BF16)
    qs_p[:, :, 0:512] = (query * expqb[:, :, None]).astype(BF16)
    qs_p[:, :, 512:514] = expqb[:, :, None].astype(BF16)
    mega[:, :, OS:OS + MT * 514] = (
        qs_p.reshape(B, MT, 128, 514).transpose(0, 2, 1, 3).reshape(B, 128, MT * 514)
    )
    # ctx [p, nt*514 + d] = [ctx | 1][nt*128 + p, d]
    ctx_p = np.empty((B, N, 514), dtype=BF16)
    ctx_p[:, :, 0:512] = ctx_bf
    ctx_p[:, :, 512:514] = BF16(1.0)
    mega[:, :, OC:OC + NT * 514] = (
        ctx_p.reshape(B, NT, 128, 514).transpose(0, 2, 1, 3).reshape(B, 128, NT * 514)
    )

    in_maps = []
    for c in range(NCORES):
        bs = slice(c * BL, (c + 1) * BL)
        aux = np.zeros((128, 40), dtype=np.float32)
        aux[:, 0:32] = (
            czc[bs].reshape(BL, NT, 128).transpose(2, 0, 1).reshape(128, BL * NT)
        )
        aux[:, 32:40] = (
            expqb[bs].reshape(BL, MT, 128).transpose(2, 0, 1).reshape(128, BL * MT)
        )
        in_maps.append({
            "ctxT": np.ascontiguousarray(mega[bs, :, OT:OT + DC * N]),
            "qTw": np.ascontiguousarray(mega[bs, :, OW:OW + DC * M]),
            "qs": np.ascontiguousarray(mega[bs, :, OS:OS + MT * 514]),
            "ctx": np.ascontiguousarray(mega[bs, :, OC:OC + NT * 514]),
            "aux": aux,
        })
    return in_maps


def run_on_device(in_maps, trace=False, repeat=1, **kw):
    from concourse.bass_utils import run_bass_kernel_spmd

    nc = get_nc(repeat)
    return run_bass_kernel_spmd(
        nc, in_maps, core_ids=list(range(NCORES)), trace=trace, **kw
    )


def _assemble(context, res_outs):
    """res_outs: list of [BL, NT, 128, 1024] bf16 per core -> full [B, N, 2048] f32."""
    dev = np.concatenate(res_outs, axis=0)                 # [B, NT, 128, 1024]
    dev = dev.reshape(B, N, 1024).astype(np.float32)
    c2q = dev[:, :, 0:512]
    q2c = dev[:, :, 512:1024]
    out = np.empty((B, N, 4 * D), dtype=np.float32)
    out[:, :, 0:512] = context
    out[:, :, 512:1024] = c2q
    out[:, :, 1024:1536] = context * c2q
    out[:, :, 1536:2048] = context * q2c
    return out


def kernel(context, query, c_mask, q_mask, w):
    context = np.ascontiguousarray(np.asarray(context, dtype=np.float32))
    in_maps = _host_prep(context, query, c_mask, q_mask, w)
    res = run_on_device(in_maps)
    return _assemble(context, [r["out"] for r in res.results])
